# revision 11
# baseline (speedup 1.0000x reference)
import sys

sys.path.insert(0, "/opt/trn_rl_repo")

import numpy as np

NCORES = 8
B, FULL_N, D = 4, 2048, 1024
NH = 16
DK = 64  # head dim
HPC = NH // NCORES  # heads per core = 2
CW = HPC * DK  # output columns per core = 128
DC = D // 128  # D chunks = 8

_CACHE = {}
LAST_RESULTS = None


def _build(n_rows):
    """SPMD Bass program for one core. Each core computes batch-0 attention
    for its 2 heads (the reference only uses att[0]) and adds it to its
    column slice of tgt for all batches.

    tgt[0]/memory[0] arrive host-transposed ([D, N]) and pre-rounded to the
    fp32r grid, declared float32r, so DMA lands matmul-ready (fp32r runs at
    full PE rate for moving dim 512). Scores are computed transposed (k on
    partitions) so softmax's P feeds P.T@V with no P transposes; V carries
    an appended ones column so the same accumulation yields softmax row
    sums. K/Q/V live in per-512-row-group tiles so attention chunks can
    start as soon as their group's projections finish."""
    import concourse.mybir as mybir
    import concourse.tile as tile
    from concourse import bacc
    from concourse.masks import make_identity

    fp32 = mybir.dt.float32
    fp32r = mybir.dt.float32r

    RT = n_rows // 128  # row tiles
    G = n_rows // 512  # 512-row groups
    QG = G
    KC = RT  # k chunks of 128

    nc = bacc.Bacc(None, target_bir_lowering=False)
    tgt0t = nc.declare_dram_parameter("tgt0t", [D, n_rows], fp32r, isOutput=False)
    mem0t = nc.declare_dram_parameter("mem0t", [D, n_rows], fp32r, isOutput=False)
    wqt = nc.declare_dram_parameter("wqt", [D, CW], fp32r, isOutput=False)
    wkt = nc.declare_dram_parameter("wkt", [D, CW], fp32r, isOutput=False)
    wvt = nc.declare_dram_parameter("wvt", [D, CW], fp32r, isOutput=False)
    tgtc = nc.declare_dram_parameter("tgtc", [B, n_rows, CW], fp32, isOutput=False)
    outc = nc.declare_dram_parameter("outc", [B, n_rows, CW], fp32, isOutput=True)

    Exp = mybir.ActivationFunctionType.Exp
    scale = 1.0 / np.sqrt(DK)

    with tile.TileContext(nc) as tc:
        with (
            tc.tile_pool(name="const", bufs=1) as const,
            tc.tile_pool(name="persist", bufs=1) as persist,
        ):
            ident = const.tile([128, 128], fp32)
            make_identity(nc, ident)

            # per-group K/Q/V tiles (fine-grained deps -> phase overlap)
            KT_gs = [
                persist.tile([128, 512], fp32r, tag=f"KT{g}", name=f"KT{g}")
                for g in range(G)
            ]
            QT_gs = [
                persist.tile([128, 512], fp32r, tag=f"QT{g}", name=f"QT{g}")
                for g in range(G)
            ]
            Vp_gs = [
                persist.tile([128, HPC, 4, DK + 1], fp32r, tag=f"Vp{g}", name=f"Vp{g}")
                for g in range(G)
            ]
            att_sb = persist.tile([128, RT, CW], fp32, tag="att")
            tgtc_sb = persist.tile([128, B, RT, CW], fp32, tag="tgtc")

            ones_f32 = const.tile([128, HPC, 4], fp32, tag="ones")
            nc.vector.memset(ones_f32, 1.0)

            # ---- Phase A: loads + QKV projections (per 512-row group) ----
            with (
                tc.tile_pool(name="wst", bufs=1) as wst_pool,
                tc.tile_pool(name="grp", bufs=1) as grp_pool,
                tc.tile_pool(name="vtg", bufs=2) as vt_pool,
                tc.tile_pool(name="ps_w", bufs=1, space="PSUM") as ps_w,
                tc.tile_pool(name="ps_acc", bufs=1, space="PSUM") as ps_acc,
            ):
                # PE warmup during the initial DMA wait (HAM un-throttle)
                for _ in range(28):
                    pw = ps_w.tile([128, 128], fp32, tag="warm")
                    nc.tensor.transpose(pw, ident, ident)

                WTs = {}
                for name, w in (("q", wqt), ("k", wkt), ("v", wvt)):
                    wt = wst_pool.tile([128, DC, CW], fp32r, tag=f"wt{name}")
                    nc.sync.dma_start(
                        out=wt, in_=w[:, :].rearrange("(c p) q -> p c q", p=128)
                    )
                    WTs[name] = wt

                for g in range(G):
                    # memory side first: feeds KT/V for this k-group
                    memT_g = grp_pool.tile([128, DC, 512], fp32r, tag="memTg")
                    for d in range(DC):
                        nc.sync.dma_start(
                            out=memT_g[:, d, :],
                            in_=mem0t[
                                d * 128 : (d + 1) * 128, g * 512 : (g + 1) * 512
                            ],
                        )
                    pk = ps_acc.tile([128, 512], fp32, tag="acc")
                    for d in range(DC):
                        nc.tensor.matmul(
                            pk, WTs["k"][:, d, :], memT_g[:, d, :],
                            start=(d == 0), stop=(d == DC - 1),
                        )
                    nc.vector.tensor_copy(out=KT_gs[g], in_=pk)
                    pv = ps_acc.tile([128, 512], fp32, tag="acc")
                    for d in range(DC):
                        nc.tensor.matmul(
                            pv, WTs["v"][:, d, :], memT_g[:, d, :],
                            start=(d == 0), stop=(d == DC - 1),
                        )
                    vt_g = vt_pool.tile([128, 512], fp32, tag="vtg")
                    nc.vector.tensor_copy(out=vt_g, in_=pv)
                    for t in range(4):
                        ptr = ps_w.tile([128, 128], fp32, tag="warm")
                        nc.tensor.transpose(ptr, vt_g[:, t * 128 : (t + 1) * 128], ident)
                        nc.vector.tensor_copy(
                            out=Vp_gs[g][:, 0, t, 0:DK], in_=ptr[:, 0:DK]
                        )
                        nc.vector.tensor_copy(
                            out=Vp_gs[g][:, 1, t, 0:DK], in_=ptr[:, DK : 2 * DK]
                        )
                    nc.vector.tensor_copy(out=Vp_gs[g][:, :, :, DK], in_=ones_f32)

                    tgtT_g = grp_pool.tile([128, DC, 512], fp32r, tag="tgtTg")
                    for d in range(DC):
                        nc.sync.dma_start(
                            out=tgtT_g[:, d, :],
                            in_=tgt0t[
                                d * 128 : (d + 1) * 128, g * 512 : (g + 1) * 512
                            ],
                        )
                    pq = ps_acc.tile([128, 512], fp32, tag="acc")
                    for d in range(DC):
                        nc.tensor.matmul(
                            pq, WTs["q"][:, d, :], tgtT_g[:, d, :],
                            start=(d == 0), stop=(d == DC - 1),
                        )
                    nc.vector.tensor_copy(out=QT_gs[g], in_=pq)

                # tgt column slices for the final broadcast add (needed late)
                for b in range(B):
                    nc.sync.dma_start(
                        out=tgtc_sb[:, b, :, :],
                        in_=tgtc[b, :, :].rearrange("(t p) c -> p t c", p=128),
                    )

                # ---- Phase B: attention per q-group, heads paired ----
                with (
                    tc.tile_pool(name="pt", bufs=1) as pt_pool,
                    tc.tile_pool(name="usb", bufs=2) as usb_pool,
                    tc.tile_pool(name="small", bufs=8) as small_pool,
                    tc.tile_pool(name="ps_st", bufs=2, space="PSUM") as ps_st,
                    tc.tile_pool(name="ps_u", bufs=2, space="PSUM") as ps_u,
                ):
                    for qg in range(QG):
                        qsl = slice(qg * 512, (qg + 1) * 512)
                        pts = [
                            pt_pool.tile(
                                [128, KC, 512], fp32r, tag=f"pt{h}", name=f"pt{h}_{qg}"
                            )
                            for h in range(HPC)
                        ]
                        pus = [
                            ps_u.tile([DK + 1, 512], fp32, tag="u", name=f"pu{h}_{qg}")
                            for h in range(HPC)
                        ]
                        for jp in range(KC // 2):
                            psts = [
                                ps_st.tile(
                                    [128, 2, 512], fp32, tag="st",
                                    name=f"st{qg}_{jp}_{h}",
                                )
                                for h in range(HPC)
                            ]
                            for jj in range(2):
                                j = jp * 2 + jj
                                kg, kt = j // 4, j % 4
                                for h in range(HPC):
                                    hs = h * DK
                                    nc.tensor.matmul(
                                        psts[h][:, jj, :],
                                        KT_gs[kg][hs : hs + DK, kt * 128 : (kt + 1) * 128],
                                        QT_gs[qg][hs : hs + DK, :],
                                        start=True, stop=True,
                                    )
                            for h in range(HPC):
                                nc.scalar.activation(
                                    out=pts[h][:, jp * 2 : jp * 2 + 2, :],
                                    in_=psts[h],
                                    func=Exp,
                                    scale=float(scale),
                                )
                            # interleave P.T@V accumulation right behind exp
                            for h in range(HPC):
                                for jj in range(2):
                                    j = jp * 2 + jj
                                    nc.tensor.matmul(
                                        pus[h],
                                        Vp_gs[j // 4][:, h, j % 4, :],
                                        pts[h][:, j, :],
                                        start=(j == 0), stop=(j == KC - 1),
                                    )
                        for h in range(HPC):
                            hs = h * DK
                            pu_sb = usb_pool.tile([DK + 1, 512], fp32, tag="usb")
                            nc.vector.tensor_copy(out=pu_sb, in_=pus[h])
                            for s in range(4):
                                pat = ps_w.tile([128, 128], fp32, tag="warm")
                                nc.tensor.transpose(
                                    pat[:, 0 : DK + 1],
                                    pu_sb[:, s * 128 : (s + 1) * 128],
                                    ident[0 : DK + 1, 0 : DK + 1],
                                )
                                rec = small_pool.tile([128, 1], fp32, tag="rec")
                                nc.vector.reciprocal(rec, pat[:, DK : DK + 1])
                                nc.vector.tensor_scalar_mul(
                                    att_sb[:, qg * 4 + s, hs : hs + DK],
                                    in0=pat[:, 0:DK],
                                    scalar1=rec,
                                )
                        # final broadcast add + store for this q-group's rows
                        for b in range(B):
                            nc.vector.tensor_add(
                                out=tgtc_sb[:, b, qg * 4 : (qg + 1) * 4, :],
                                in0=tgtc_sb[:, b, qg * 4 : (qg + 1) * 4, :],
                                in1=att_sb[:, qg * 4 : (qg + 1) * 4, :],
                            )
                            nc.sync.dma_start(
                                out=outc[b, qsl, :].rearrange(
                                    "(t p) c -> p t c", p=128
                                ),
                                in_=tgtc_sb[:, b, qg * 4 : (qg + 1) * 4, :],
                            )

    nc.finalize()
    return nc


def _get_nc(n_rows):
    if n_rows not in _CACHE:
        _CACHE[n_rows] = _build(n_rows)
    return _CACHE[n_rows]


def _round_fp32r(x):
    """Round fp32 to the fp32r grid (11 explicit mantissa bits, RNE)."""
    v = np.ascontiguousarray(x, dtype=np.float32).view(np.uint32)
    lo = v & np.uint32(0xFFF)
    base = v & ~np.uint32(0xFFF)
    lsb = (v >> np.uint32(12)) & np.uint32(1)
    up = (lo > 0x800) | ((lo == 0x800) & (lsb == 1))
    out = base + (up.astype(np.uint32) << np.uint32(12))
    return out.view(np.float32)


def _run(tgt, memory, Wq, Wk, Wv, trace=False):
    global LAST_RESULTS
    from concourse.bass_utils import run_bass_kernel_spmd

    n_rows = tgt.shape[1]
    nc = _get_nc(n_rows)

    tgt = np.ascontiguousarray(tgt, dtype=np.float32)
    memory = np.ascontiguousarray(memory, dtype=np.float32)
    tgt0t = _round_fp32r(np.ascontiguousarray(tgt[0].T))
    mem0t = _round_fp32r(np.ascontiguousarray(memory[0].T))

    in_maps = []
    for c in range(NCORES):
        sl = slice(c * CW, (c + 1) * CW)
        in_maps.append(
            {
                "tgt0t": tgt0t,
                "mem0t": mem0t,
                "wqt": _round_fp32r(Wq[sl, :].T),
                "wkt": _round_fp32r(Wk[sl, :].T),
                "wvt": _round_fp32r(Wv[sl, :].T),
                "tgtc": np.ascontiguousarray(tgt[:, :, sl]),
            }
        )
    res = run_bass_kernel_spmd(nc, in_maps, list(range(NCORES)), trace=trace)
    LAST_RESULTS = res
    out = np.concatenate([res.results[c]["outc"] for c in range(NCORES)], axis=2)
    return out


def kernel(tgt, memory, Wq, Wk, Wv):
    return _run(tgt, memory, Wq, Wk, Wv)


# revision 12
# speedup vs baseline: 1.0794x; 1.0794x over previous
import sys

sys.path.insert(0, "/opt/trn_rl_repo")

import numpy as np

NCORES = 8
B, FULL_N, D = 4, 2048, 1024
NH = 16
DK = 64  # head dim
HPC = NH // NCORES  # heads per core = 2
CW = HPC * DK  # output columns per core = 128
DC = D // 128  # D chunks = 8

_CACHE = {}
LAST_RESULTS = None


def _build(n_rows):
    """SPMD Bass program for one core. Each core computes batch-0 attention
    for its 2 heads (the reference only uses att[0]) and adds it to its
    column slice of tgt for all batches.

    tgt[0]/memory[0] arrive host-transposed ([D, N]) and pre-rounded to the
    fp32r grid, declared float32r, so DMA lands matmul-ready (fp32r runs at
    full PE rate for moving dim 512). Scores are computed transposed (k on
    partitions) so softmax's P feeds P.T@V with no P transposes; V carries
    an appended ones column so the same accumulation yields softmax row
    sums. K/Q/V live in per-512-row-group tiles so attention chunks can
    start as soon as their group's projections finish."""
    import concourse.mybir as mybir
    import concourse.tile as tile
    from concourse import bacc
    from concourse.masks import make_identity

    fp32 = mybir.dt.float32
    fp32r = mybir.dt.float32r

    RT = n_rows // 128  # row tiles
    G = n_rows // 512  # 512-row groups
    QG = G
    KC = RT  # k chunks of 128

    nc = bacc.Bacc(None, target_bir_lowering=False)
    tgt0t = nc.declare_dram_parameter("tgt0t", [D, n_rows], fp32r, isOutput=False)
    mem0t = nc.declare_dram_parameter("mem0t", [D, n_rows], fp32r, isOutput=False)
    wqt = nc.declare_dram_parameter("wqt", [D, CW], fp32r, isOutput=False)
    wkt = nc.declare_dram_parameter("wkt", [D, CW], fp32r, isOutput=False)
    wvt = nc.declare_dram_parameter("wvt", [D, CW], fp32r, isOutput=False)
    tgtc = nc.declare_dram_parameter("tgtc", [B, n_rows, CW], fp32, isOutput=False)
    outc = nc.declare_dram_parameter("outc", [B, n_rows, CW], fp32, isOutput=True)

    Exp = mybir.ActivationFunctionType.Exp
    scale = 1.0 / np.sqrt(DK)

    with tile.TileContext(nc) as tc:
        with (
            tc.tile_pool(name="const", bufs=1) as const,
            tc.tile_pool(name="persist", bufs=1) as persist,
        ):
            ident = const.tile([128, 128], fp32)
            make_identity(nc, ident)

            # per-group K/Q/V tiles (fine-grained deps -> phase overlap)
            KT_gs = [
                persist.tile([128, 512], fp32r, tag=f"KT{g}", name=f"KT{g}")
                for g in range(G)
            ]
            QT_gs = [
                persist.tile([128, 512], fp32r, tag=f"QT{g}", name=f"QT{g}")
                for g in range(G)
            ]
            Vp_gs = [
                persist.tile([128, HPC, 4, DK + 1], fp32r, tag=f"Vp{g}", name=f"Vp{g}")
                for g in range(G)
            ]
            att_sb = persist.tile([128, RT, CW], fp32, tag="att")
            tgtc_sb = persist.tile([128, B, RT, CW], fp32, tag="tgtc")

            ones_f32 = const.tile([128, HPC, 4], fp32, tag="ones")
            nc.vector.memset(ones_f32, 1.0)

            # ---- Phase A: loads + QKV projections (per 512-row group) ----
            with (
                tc.tile_pool(name="wst", bufs=1) as wst_pool,
                tc.tile_pool(name="grp", bufs=1) as grp_pool,
                tc.tile_pool(name="vtg", bufs=2) as vt_pool,
                tc.tile_pool(name="ps_w", bufs=1, space="PSUM") as ps_w,
                tc.tile_pool(name="ps_acc", bufs=2, space="PSUM") as ps_acc,
            ):
                # PE warmup during the initial DMA wait (HAM un-throttle)
                for _ in range(16):
                    pw = ps_w.tile([128, 128], fp32, tag="warm")
                    nc.tensor.transpose(pw, ident, ident)

                WTs = {}
                for name, w in (("q", wqt), ("k", wkt), ("v", wvt)):
                    wt = wst_pool.tile([128, DC, CW], fp32r, tag=f"wt{name}")
                    nc.sync.dma_start(
                        out=wt, in_=w[:, :].rearrange("(c p) q -> p c q", p=128)
                    )
                    WTs[name] = wt

                def emit_mem_group(g):
                    memT_g = grp_pool.tile(
                        [128, DC, 512], fp32r, tag="memTg", name=f"memT{g}"
                    )
                    for d in range(DC):
                        nc.sync.dma_start(
                            out=memT_g[:, d, :],
                            in_=mem0t[
                                d * 128 : (d + 1) * 128, g * 512 : (g + 1) * 512
                            ],
                        )
                    pk = ps_acc.tile([128, 512], fp32, tag="acc")
                    for d in range(DC):
                        nc.tensor.matmul(
                            pk, WTs["k"][:, d, :], memT_g[:, d, :],
                            start=(d == 0), stop=(d == DC - 1),
                        )
                    nc.vector.tensor_copy(out=KT_gs[g], in_=pk)
                    pv = ps_acc.tile([128, 512], fp32, tag="acc")
                    for d in range(DC):
                        nc.tensor.matmul(
                            pv, WTs["v"][:, d, :], memT_g[:, d, :],
                            start=(d == 0), stop=(d == DC - 1),
                        )
                    vt_g = vt_pool.tile([128, 512], fp32, tag="vtg")
                    nc.vector.tensor_copy(out=vt_g, in_=pv)
                    for t in range(4):
                        ptr = ps_w.tile([128, 128], fp32, tag="warm")
                        nc.tensor.transpose(ptr, vt_g[:, t * 128 : (t + 1) * 128], ident)
                        nc.vector.tensor_copy(
                            out=Vp_gs[g][:, 0, t, 0:DK], in_=ptr[:, 0:DK]
                        )
                        nc.vector.tensor_copy(
                            out=Vp_gs[g][:, 1, t, 0:DK], in_=ptr[:, DK : 2 * DK]
                        )
                    nc.vector.tensor_copy(out=Vp_gs[g][:, :, :, DK], in_=ones_f32)

                def emit_tgt_group(g):
                    tgtT_g = grp_pool.tile(
                        [128, DC, 512], fp32r, tag="tgtTg", name=f"tgtT{g}"
                    )
                    for d in range(DC):
                        nc.sync.dma_start(
                            out=tgtT_g[:, d, :],
                            in_=tgt0t[
                                d * 128 : (d + 1) * 128, g * 512 : (g + 1) * 512
                            ],
                        )
                    pq = ps_acc.tile([128, 512], fp32, tag="acc")
                    for d in range(DC):
                        nc.tensor.matmul(
                            pq, WTs["q"][:, d, :], tgtT_g[:, d, :],
                            start=(d == 0), stop=(d == DC - 1),
                        )
                    nc.vector.tensor_copy(out=QT_gs[g], in_=pq)

                # all memory-side groups first (attention needs full K/V),
                # then the first q projection; remaining q projections are
                # emitted inside the qg loop as PE filler
                for g in range(G):
                    emit_mem_group(g)
                emit_tgt_group(0)

                for b in range(B):
                    nc.sync.dma_start(
                        out=tgtc_sb[:, b, :, :],
                        in_=tgtc[b, :, :].rearrange("(t p) c -> p t c", p=128),
                    )

                # ---- Phase B: attention per q-group, heads paired ----
                with (
                    tc.tile_pool(name="pt", bufs=1) as pt_pool,
                    tc.tile_pool(name="usb", bufs=2) as usb_pool,
                    tc.tile_pool(name="small", bufs=8) as small_pool,
                    tc.tile_pool(name="ps_st", bufs=2, space="PSUM") as ps_st,
                    tc.tile_pool(name="ps_u", bufs=1, space="PSUM") as ps_u,
                ):
                    for qg in range(QG):
                        if qg + 1 < QG:
                            emit_tgt_group(qg + 1)
                        qsl = slice(qg * 512, (qg + 1) * 512)
                        pts = [
                            pt_pool.tile(
                                [128, KC, 512], fp32r, tag=f"pt{h}", name=f"pt{h}_{qg}"
                            )
                            for h in range(HPC)
                        ]
                        for jp in range(KC // 2):
                            psts = [
                                ps_st.tile(
                                    [128, 2, 512], fp32, tag="st",
                                    name=f"st{qg}_{jp}_{h}",
                                )
                                for h in range(HPC)
                            ]
                            for jj in range(2):
                                j = jp * 2 + jj
                                kg, kt = j // 4, j % 4
                                for h in range(HPC):
                                    hs = h * DK
                                    nc.tensor.matmul(
                                        psts[h][:, jj, :],
                                        KT_gs[kg][hs : hs + DK, kt * 128 : (kt + 1) * 128],
                                        QT_gs[qg][hs : hs + DK, :],
                                        start=True, stop=True,
                                    )
                            for h in range(HPC):
                                nc.scalar.activation(
                                    out=pts[h][:, jp * 2 : jp * 2 + 2, :],
                                    in_=psts[h],
                                    func=Exp,
                                    scale=float(scale),
                                )
                        for h in range(HPC):
                            hs = h * DK
                            pu = ps_u.tile([DK + 1, 512], fp32, tag="u")
                            for j in range(KC):
                                nc.tensor.matmul(
                                    pu,
                                    Vp_gs[j // 4][:, h, j % 4, :],
                                    pts[h][:, j, :],
                                    start=(j == 0), stop=(j == KC - 1),
                                )
                            pu_sb = usb_pool.tile([DK + 1, 512], fp32, tag="usb")
                            nc.vector.tensor_copy(out=pu_sb, in_=pu)
                            for s in range(4):
                                pat = ps_w.tile([128, 128], fp32, tag="warm")
                                nc.tensor.transpose(
                                    pat[:, 0 : DK + 1],
                                    pu_sb[:, s * 128 : (s + 1) * 128],
                                    ident[0 : DK + 1, 0 : DK + 1],
                                )
                                rec = small_pool.tile([128, 1], fp32, tag="rec")
                                nc.vector.reciprocal(rec, pat[:, DK : DK + 1])
                                nc.vector.tensor_scalar_mul(
                                    att_sb[:, qg * 4 + s, hs : hs + DK],
                                    in0=pat[:, 0:DK],
                                    scalar1=rec,
                                )
                        # final broadcast add + store for this q-group's rows
                        for b in range(B):
                            nc.vector.tensor_add(
                                out=tgtc_sb[:, b, qg * 4 : (qg + 1) * 4, :],
                                in0=tgtc_sb[:, b, qg * 4 : (qg + 1) * 4, :],
                                in1=att_sb[:, qg * 4 : (qg + 1) * 4, :],
                            )
                            nc.sync.dma_start(
                                out=outc[b, qsl, :].rearrange(
                                    "(t p) c -> p t c", p=128
                                ),
                                in_=tgtc_sb[:, b, qg * 4 : (qg + 1) * 4, :],
                            )

    nc.finalize()
    return nc


def _get_nc(n_rows):
    if n_rows not in _CACHE:
        _CACHE[n_rows] = _build(n_rows)
    return _CACHE[n_rows]


def _round_fp32r(x):
    """Round fp32 to the fp32r grid (11 explicit mantissa bits, RNE)."""
    v = np.ascontiguousarray(x, dtype=np.float32).view(np.uint32)
    lo = v & np.uint32(0xFFF)
    base = v & ~np.uint32(0xFFF)
    lsb = (v >> np.uint32(12)) & np.uint32(1)
    up = (lo > 0x800) | ((lo == 0x800) & (lsb == 1))
    out = base + (up.astype(np.uint32) << np.uint32(12))
    return out.view(np.float32)


def _run(tgt, memory, Wq, Wk, Wv, trace=False):
    global LAST_RESULTS
    from concourse.bass_utils import run_bass_kernel_spmd

    n_rows = tgt.shape[1]
    nc = _get_nc(n_rows)

    tgt = np.ascontiguousarray(tgt, dtype=np.float32)
    memory = np.ascontiguousarray(memory, dtype=np.float32)
    tgt0t = _round_fp32r(np.ascontiguousarray(tgt[0].T))
    mem0t = _round_fp32r(np.ascontiguousarray(memory[0].T))

    in_maps = []
    for c in range(NCORES):
        sl = slice(c * CW, (c + 1) * CW)
        in_maps.append(
            {
                "tgt0t": tgt0t,
                "mem0t": mem0t,
                "wqt": _round_fp32r(Wq[sl, :].T),
                "wkt": _round_fp32r(Wk[sl, :].T),
                "wvt": _round_fp32r(Wv[sl, :].T),
                "tgtc": np.ascontiguousarray(tgt[:, :, sl]),
            }
        )
    res = run_bass_kernel_spmd(nc, in_maps, list(range(NCORES)), trace=trace)
    LAST_RESULTS = res
    out = np.concatenate([res.results[c]["outc"] for c in range(NCORES)], axis=2)
    return out


def kernel(tgt, memory, Wq, Wk, Wv):
    return _run(tgt, memory, Wq, Wk, Wv)


# revision 13
# speedup vs baseline: 1.2490x; 1.1570x over previous
import sys

sys.path.insert(0, "/opt/trn_rl_repo")

import numpy as np

NCORES = 8
B, FULL_N, D = 4, 2048, 1024
NH = 16
DK = 64  # head dim
HPC = NH // NCORES  # heads per core = 2
CW = HPC * DK  # output columns per core = 128
DC = D // 128  # D chunks = 8

_CACHE = {}
LAST_RESULTS = None


def _build(n_rows):
    """SPMD Bass program for one core. Each core computes batch-0 attention
    for its 2 heads (the reference only uses att[0]) and adds it to its
    column slice of tgt for all batches.

    tgt[0]/memory[0] arrive host-transposed ([D, N]) and pre-rounded to the
    fp32r grid, declared float32r, so DMA lands matmul-ready (fp32r runs at
    full PE rate for moving dim 512). Scores are computed transposed (k on
    partitions) so softmax's P feeds P.T@V with no P transposes; V carries
    an appended ones column so the same accumulation yields softmax row
    sums. K/Q/V live in per-512-row-group tiles so attention chunks can
    start as soon as their group's projections finish."""
    import concourse.mybir as mybir
    import concourse.tile as tile
    from concourse import bacc
    from concourse.masks import make_identity

    fp32 = mybir.dt.float32
    fp32r = mybir.dt.float32r
    bf16 = mybir.dt.bfloat16

    RT = n_rows // 128  # row tiles
    G = n_rows // 512  # 512-row groups
    QG = G
    KC = RT  # k chunks of 128

    nc = bacc.Bacc(None, target_bir_lowering=False)
    tgt0t = nc.declare_dram_parameter("tgt0t", [D, n_rows], fp32r, isOutput=False)
    mem0t = nc.declare_dram_parameter("mem0t", [D, n_rows], fp32r, isOutput=False)
    wqt = nc.declare_dram_parameter("wqt", [D, CW], fp32r, isOutput=False)
    wkt = nc.declare_dram_parameter("wkt", [D, CW], fp32r, isOutput=False)
    wvt = nc.declare_dram_parameter("wvt", [D, CW], fp32r, isOutput=False)
    tgtc = nc.declare_dram_parameter("tgtc", [B, n_rows, CW], fp32, isOutput=False)
    outc = nc.declare_dram_parameter("outc", [B, n_rows, CW], fp32, isOutput=True)

    Exp = mybir.ActivationFunctionType.Exp
    scale = 1.0 / np.sqrt(DK)

    with tile.TileContext(nc) as tc:
        with (
            tc.tile_pool(name="const", bufs=1) as const,
            tc.tile_pool(name="persist", bufs=1) as persist,
        ):
            ident = const.tile([128, 128], fp32)
            make_identity(nc, ident)

            # per-group K/Q/V tiles (fine-grained deps -> phase overlap)
            KT_gs = [
                persist.tile([128, 512], fp32r, tag=f"KT{g}", name=f"KT{g}")
                for g in range(G)
            ]
            QT_gs = [
                persist.tile([128, 512], fp32r, tag=f"QT{g}", name=f"QT{g}")
                for g in range(G)
            ]
            Vp_gs = [
                persist.tile([128, HPC, 4, DK + 1], bf16, tag=f"Vp{g}", name=f"Vp{g}")
                for g in range(G)
            ]
            att_sb = persist.tile([128, RT, CW], fp32, tag="att")
            tgtc_sb = persist.tile([128, B, RT, CW], fp32, tag="tgtc")

            ones_f32 = const.tile([128, HPC, 4], fp32, tag="ones")
            nc.vector.memset(ones_f32, 1.0)

            # ---- Phase A: loads + QKV projections (per 512-row group) ----
            with (
                tc.tile_pool(name="wst", bufs=1) as wst_pool,
                tc.tile_pool(name="grp", bufs=2) as grp_pool,
                tc.tile_pool(name="vtg", bufs=2) as vt_pool,
                tc.tile_pool(name="ps_w", bufs=1, space="PSUM") as ps_w,
                tc.tile_pool(name="ps_acc", bufs=2, space="PSUM") as ps_acc,
            ):
                # PE warmup during the initial DMA wait (HAM un-throttle)
                for _ in range(16):
                    pw = ps_w.tile([128, 128], fp32, tag="warm")
                    nc.tensor.transpose(pw, ident, ident)

                WTs = {}
                for name, w in (("q", wqt), ("k", wkt), ("v", wvt)):
                    wt = wst_pool.tile([128, DC, CW], fp32r, tag=f"wt{name}")
                    nc.sync.dma_start(
                        out=wt, in_=w[:, :].rearrange("(c p) q -> p c q", p=128)
                    )
                    WTs[name] = wt

                def emit_mem_group(g):
                    memT_g = grp_pool.tile(
                        [128, DC, 512], fp32r, tag="memTg", name=f"memT{g}"
                    )
                    for d in range(DC):
                        nc.sync.dma_start(
                            out=memT_g[:, d, :],
                            in_=mem0t[
                                d * 128 : (d + 1) * 128, g * 512 : (g + 1) * 512
                            ],
                        )
                    pk = ps_acc.tile([128, 512], fp32, tag="acc")
                    for d in range(DC):
                        nc.tensor.matmul(
                            pk, WTs["k"][:, d, :], memT_g[:, d, :],
                            start=(d == 0), stop=(d == DC - 1),
                        )
                    nc.vector.tensor_copy(out=KT_gs[g], in_=pk)
                    pv = ps_acc.tile([128, 512], fp32, tag="acc")
                    for d in range(DC):
                        nc.tensor.matmul(
                            pv, WTs["v"][:, d, :], memT_g[:, d, :],
                            start=(d == 0), stop=(d == DC - 1),
                        )
                    vt_g = vt_pool.tile([128, 512], fp32, tag="vtg")
                    nc.vector.tensor_copy(out=vt_g, in_=pv)
                    for t in range(4):
                        ptr = ps_w.tile([128, 128], fp32, tag="warm")
                        nc.tensor.transpose(ptr, vt_g[:, t * 128 : (t + 1) * 128], ident)
                        nc.vector.tensor_copy(
                            out=Vp_gs[g][:, 0, t, 0:DK], in_=ptr[:, 0:DK]
                        )
                        nc.vector.tensor_copy(
                            out=Vp_gs[g][:, 1, t, 0:DK], in_=ptr[:, DK : 2 * DK]
                        )
                    nc.vector.tensor_copy(out=Vp_gs[g][:, :, :, DK], in_=ones_f32)

                def emit_tgt_group(g):
                    tgtT_g = grp_pool.tile(
                        [128, DC, 512], fp32r, tag="tgtTg", name=f"tgtT{g}"
                    )
                    for d in range(DC):
                        nc.sync.dma_start(
                            out=tgtT_g[:, d, :],
                            in_=tgt0t[
                                d * 128 : (d + 1) * 128, g * 512 : (g + 1) * 512
                            ],
                        )
                    pq = ps_acc.tile([128, 512], fp32, tag="acc")
                    for d in range(DC):
                        nc.tensor.matmul(
                            pq, WTs["q"][:, d, :], tgtT_g[:, d, :],
                            start=(d == 0), stop=(d == DC - 1),
                        )
                    nc.vector.tensor_copy(out=QT_gs[g], in_=pq)

                # all memory-side groups first (attention needs full K/V),
                # then the first q projection; remaining q projections are
                # emitted inside the qg loop as PE filler
                for g in range(G):
                    emit_mem_group(g)
                emit_tgt_group(0)

                for b in range(B):
                    nc.sync.dma_start(
                        out=tgtc_sb[:, b, :, :],
                        in_=tgtc[b, :, :].rearrange("(t p) c -> p t c", p=128),
                    )

                # ---- Phase B: attention per q-group, heads paired ----
                with (
                    tc.tile_pool(name="pt", bufs=1) as pt_pool,
                    tc.tile_pool(name="usb", bufs=2) as usb_pool,
                    tc.tile_pool(name="small", bufs=8) as small_pool,
                    tc.tile_pool(name="ps_st", bufs=2, space="PSUM") as ps_st,
                    tc.tile_pool(name="ps_u", bufs=1, space="PSUM") as ps_u,
                ):
                    for qg in range(QG):
                        if qg + 1 < QG:
                            emit_tgt_group(qg + 1)
                        qsl = slice(qg * 512, (qg + 1) * 512)
                        pts = [
                            pt_pool.tile(
                                [128, KC, 512], bf16, tag=f"pt{h}", name=f"pt{h}_{qg}"
                            )
                            for h in range(HPC)
                        ]
                        for jp in range(KC // 2):
                            psts = [
                                ps_st.tile(
                                    [128, 2, 512], fp32, tag="st",
                                    name=f"st{qg}_{jp}_{h}",
                                )
                                for h in range(HPC)
                            ]
                            for jj in range(2):
                                j = jp * 2 + jj
                                kg, kt = j // 4, j % 4
                                for h in range(HPC):
                                    hs = h * DK
                                    nc.tensor.matmul(
                                        psts[h][:, jj, :],
                                        KT_gs[kg][hs : hs + DK, kt * 128 : (kt + 1) * 128],
                                        QT_gs[qg][hs : hs + DK, :],
                                        start=True, stop=True,
                                    )
                            for h in range(HPC):
                                nc.scalar.activation(
                                    out=pts[h][:, jp * 2 : jp * 2 + 2, :],
                                    in_=psts[h],
                                    func=Exp,
                                    scale=float(scale),
                                )
                        for h in range(HPC):
                            hs = h * DK
                            pu = ps_u.tile([DK + 1, 512], fp32, tag="u")
                            for j in range(KC):
                                nc.tensor.matmul(
                                    pu,
                                    Vp_gs[j // 4][:, h, j % 4, :],
                                    pts[h][:, j, :],
                                    start=(j == 0), stop=(j == KC - 1),
                                )
                            pu_sb = usb_pool.tile([DK + 1, 512], fp32, tag="usb")
                            nc.vector.tensor_copy(out=pu_sb, in_=pu)
                            for s in range(4):
                                pat = ps_w.tile([128, 128], fp32, tag="warm")
                                nc.tensor.transpose(
                                    pat[:, 0 : DK + 1],
                                    pu_sb[:, s * 128 : (s + 1) * 128],
                                    ident[0 : DK + 1, 0 : DK + 1],
                                )
                                rec = small_pool.tile([128, 1], fp32, tag="rec")
                                nc.vector.reciprocal(rec, pat[:, DK : DK + 1])
                                nc.vector.tensor_scalar_mul(
                                    att_sb[:, qg * 4 + s, hs : hs + DK],
                                    in0=pat[:, 0:DK],
                                    scalar1=rec,
                                )
                        # final broadcast add + store for this q-group's rows
                        for b in range(B):
                            nc.vector.tensor_add(
                                out=tgtc_sb[:, b, qg * 4 : (qg + 1) * 4, :],
                                in0=tgtc_sb[:, b, qg * 4 : (qg + 1) * 4, :],
                                in1=att_sb[:, qg * 4 : (qg + 1) * 4, :],
                            )
                            nc.sync.dma_start(
                                out=outc[b, qsl, :].rearrange(
                                    "(t p) c -> p t c", p=128
                                ),
                                in_=tgtc_sb[:, b, qg * 4 : (qg + 1) * 4, :],
                            )

    nc.finalize()
    return nc


def _get_nc(n_rows):
    if n_rows not in _CACHE:
        _CACHE[n_rows] = _build(n_rows)
    return _CACHE[n_rows]


def _round_fp32r(x):
    """Round fp32 to the fp32r grid (11 explicit mantissa bits, RNE)."""
    v = np.ascontiguousarray(x, dtype=np.float32).view(np.uint32)
    lo = v & np.uint32(0xFFF)
    base = v & ~np.uint32(0xFFF)
    lsb = (v >> np.uint32(12)) & np.uint32(1)
    up = (lo > 0x800) | ((lo == 0x800) & (lsb == 1))
    out = base + (up.astype(np.uint32) << np.uint32(12))
    return out.view(np.float32)


def _run(tgt, memory, Wq, Wk, Wv, trace=False):
    global LAST_RESULTS
    from concourse.bass_utils import run_bass_kernel_spmd

    n_rows = tgt.shape[1]
    nc = _get_nc(n_rows)

    tgt = np.ascontiguousarray(tgt, dtype=np.float32)
    memory = np.ascontiguousarray(memory, dtype=np.float32)
    tgt0t = _round_fp32r(np.ascontiguousarray(tgt[0].T))
    mem0t = _round_fp32r(np.ascontiguousarray(memory[0].T))

    in_maps = []
    for c in range(NCORES):
        sl = slice(c * CW, (c + 1) * CW)
        in_maps.append(
            {
                "tgt0t": tgt0t,
                "mem0t": mem0t,
                "wqt": _round_fp32r(Wq[sl, :].T),
                "wkt": _round_fp32r(Wk[sl, :].T),
                "wvt": _round_fp32r(Wv[sl, :].T),
                "tgtc": np.ascontiguousarray(tgt[:, :, sl]),
            }
        )
    res = run_bass_kernel_spmd(nc, in_maps, list(range(NCORES)), trace=trace)
    LAST_RESULTS = res
    out = np.concatenate([res.results[c]["outc"] for c in range(NCORES)], axis=2)
    return out


def kernel(tgt, memory, Wq, Wk, Wv):
    return _run(tgt, memory, Wq, Wk, Wv)
